# revision 13
# baseline (speedup 1.0000x reference)
"""GatedGraphConv (3-layer, GRU) Bass kernel for 8 Trainium2 NeuronCores. v3.

Strategy:
  - Shard nodes (dst segments) across 8 cores; each core owns 12500 dst nodes
    placed into 12800 padded positions via a host-chosen PERMUTATION that
    load-balances edges across (window, group, superblock) cells so every
    cell holds <= T*128 edges (T=4) with minimal padding.
  - fp32 data path end-to-end (the grading metric's +1e-3 denominator floor
    demands ~1e-5 absolute accuracy; 16-bit anywhere fails).
  - Messages gathered with ONE dma_gather per (window, superblock) =
    100 calls/layer of 2048 rows each (129 descs/engine-ring, under the 256
    ring capacity), spread across all 4 SWDGE queues so descriptor
    generation runs on all 8 GpSimd Q7 cores concurrently.
  - All padded slots gather row 0 with edge-weight 0 (no cnt registers).
  - Segment sums via TensorE matmuls with on-the-fly one-hot S matrices:
    S[slot, j] = ew[slot] * (dst_rel[slot] == j), accumulated in fp32 PSUM.
  - GRU per 512-node chunk interleaved with aggregation windows; elementwise
    split across ACT (sigmoid/tanh/copies/bias-adds) and DVE.
  - W_l folded into W_ih on the host (Wi_eff = W_ih @ W_l.T).
  - Updated h transposed back to row-major (PE), written to a bounce buffer,
    AllGathered so every core holds the full h table for the next layer.
  - Final layer writes fp32 permuted rows; host un-permutes.
"""

import sys
import numpy as np

for _p in ("/opt/trn_rl_repo",):
    if _p not in sys.path:
        sys.path.append(_p)

# ---------------------------------------------------------------------------
# constants (hardcoded problem shape)
# ---------------------------------------------------------------------------
N = 100000          # nodes
D = 128             # feature dim
L = 3               # layers
C = 8               # cores
NC_ = N // C        # real nodes per core (12500)
NCP = 12800         # padded positions per core
WIN = 512           # dst nodes per PSUM window
NW = NCP // WIN     # windows per core (25)
G = WIN // 128      # 128-wide subgroups per window (4)
SB = 4              # superblocks of gather table (int16 index limit)
TROWS = C * NCP     # padded gather-table rows (102400)
SBROWS = TROWS // SB  # rows per superblock (25600)
NQUEUES = 4         # SWDGE queues for parallel gather desc-gen
GSPLIT = 2          # gathers per (window, superblock) unit
NCELLC = NW * G * SB  # cells per core (400)
NBIN = NW * G       # dst bins per core (100)


def _ceil_div(a, b):
    return -(-a // b)


# ---------------------------------------------------------------------------
# host-side planning
# ---------------------------------------------------------------------------
def _plan(edge_index, edge_attr):
    """Balance dst nodes into bins, build gather indices + S-build scalars."""
    src = np.asarray(edge_index[0], dtype=np.int64)
    dst = np.asarray(edge_index[1], dtype=np.int64)
    ew = np.asarray(edge_attr, dtype=np.float32)
    E = src.shape[0]

    sb_e = src // (2 * NC_)  # source superblock, permutation-independent

    # per-(dst node, sb) edge counts
    cnt_ns = np.bincount(dst * SB + sb_e, minlength=N * SB).reshape(N, SB)

    # --- greedy multi-dim balance of dst nodes into NBIN bins per core ---
    pos = np.empty(N, dtype=np.int64)  # position of each node within its core
    maxcell = 0
    for c in range(C):
        v = cnt_ns[c * NC_:(c + 1) * NC_]  # [12500, 4]
        order = np.argsort(-v.sum(1), kind="stable")
        load = np.zeros((NBIN, SB), dtype=np.int64)
        nn = np.zeros(NBIN, dtype=np.int64)
        binof = np.empty(NC_, dtype=np.int64)
        for j in order:
            score = (load + v[j]).max(1).astype(np.float64) + nn * 1e-3
            score[nn >= 128] = np.inf
            b = int(np.argmin(score))
            binof[j] = b
            load[b] += v[j]
            nn[b] += 1
        o2 = np.argsort(binof, kind="stable")
        bsort = binof[o2]
        cnts = np.bincount(bsort, minlength=NBIN)
        starts = np.concatenate(([0], np.cumsum(cnts)[:-1]))
        ranks = np.arange(NC_) - starts[bsort]
        pl = np.empty(NC_, dtype=np.int64)
        pl[o2] = bsort * 128 + ranks
        pos[c * NC_:(c + 1) * NC_] = pl
        maxcell = max(maxcell, int(load.max()))

    T = _ceil_div(max(1, maxcell), 128)
    spc_w = G * T * 128  # slots per (window, sb) gather unit

    # --- per-edge slot assignment ---
    dc = dst // NC_
    posd = pos[dst]
    w = posd // WIN
    g = (posd % WIN) // 128
    rel = (posd % 128).astype(np.float32)
    srow_local = (src // NC_) * NCP + pos[src] - sb_e * SBROWS  # [0, 25600)

    cell = ((dc * NW + w) * G + g) * SB + sb_e
    order_e = np.argsort(cell, kind="stable")
    cell_s = cell[order_e]
    ccnt = np.bincount(cell_s, minlength=C * NCELLC)
    starts = np.concatenate(([0], np.cumsum(ccnt)[:-1]))
    rank = np.arange(E) - starts[cell_s]
    rank_e = np.empty(E, dtype=np.int64)
    rank_e[order_e] = rank
    assert int(ccnt.max()) <= T * 128

    # unit (w, sb) at slot base (w*SB + sb) * spc_w; in-unit slot = g*T*128+rank
    slot = (w * SB + sb_e) * spc_w + g * T * 128 + rank_e
    IDXCOLS = NW * SB * spc_w // 16

    idx16 = np.zeros((C, 16, IDXCOLS), dtype=np.int16)
    idx16[dc, slot % 16, slot // 16] = srow_local.astype(np.int16)

    t_e = rank_e // 128
    p_e = rank_e % 128
    colc = ((w * G + g) * SB + sb_e) * T + t_e
    relcols = np.zeros((C, 128, NCELLC * T), dtype=np.float32)
    ewcols = np.zeros((C, 128, NCELLC * T), dtype=np.float32)
    relcols[dc, p_e, colc] = rel
    ewcols[dc, p_e, colc] = ew

    return T, idx16, relcols, ewcols, pos


# ---------------------------------------------------------------------------
# device program
# ---------------------------------------------------------------------------
def _build_program(T):
    from contextlib import ExitStack
    import concourse.bass as bass
    import concourse.tile as tile
    from concourse import bacc, mybir

    f32 = mybir.dt.float32
    i16 = mybir.dt.int16
    eq = mybir.AluOpType.is_equal
    mult = mybir.AluOpType.mult
    AF = mybir.ActivationFunctionType

    spc_w = G * T * 128
    IDXCOLS = NW * SB * spc_w // 16

    nc = bacc.Bacc("TRN2", target_bir_lowering=False, debug=False,
                   num_devices=C, num_swdge_queues=NQUEUES)

    x_tab = nc.dram_tensor("x_tab", [TROWS, D], f32, kind="ExternalInput").ap()
    x_ownT = nc.dram_tensor("x_ownT", [D, NCP], f32, kind="ExternalInput").ap()
    idx_dram = nc.dram_tensor("idx_dram", [128, IDXCOLS], i16,
                              kind="ExternalInput").ap()
    rel_dram = nc.dram_tensor("rel_dram", [128, NCELLC * T], f32,
                              kind="ExternalInput").ap()
    ewc_dram = nc.dram_tensor("ewc_dram", [128, NCELLC * T], f32,
                              kind="ExternalInput").ap()
    wie_dram = nc.dram_tensor("wie_dram", [128, L * 3 * 128], f32, kind="ExternalInput").ap()
    whh_dram = nc.dram_tensor("whh_dram", [128, 3 * 128], f32, kind="ExternalInput").ap()
    bias_dram = nc.dram_tensor("bias_dram", [128, 4], f32, kind="ExternalInput").ap()
    iota_dram = nc.dram_tensor("iota_dram", [128, 128], f32, kind="ExternalInput").ap()
    ident_dram = nc.dram_tensor("ident_dram", [128, 128], f32, kind="ExternalInput").ap()

    out = nc.dram_tensor("out", [NCP, D], f32, kind="ExternalOutput").ap()

    with tile.TileContext(nc) as tc, ExitStack() as ctx:
        const = ctx.enter_context(tc.tile_pool(name="const", bufs=1))
        dram = ctx.enter_context(tc.tile_pool(name="dram", bufs=1, space="DRAM"))
        idxp = ctx.enter_context(tc.tile_pool(name="idxp", bufs=2))
        sp = ctx.enter_context(tc.tile_pool(name="sp", bufs=4))
        aggps = ctx.enter_context(tc.tile_pool(name="aggps", bufs=2, space="PSUM"))
        grups = ctx.enter_context(tc.tile_pool(name="grups", bufs=1, space="PSUM"))
        aggsb = ctx.enter_context(tc.tile_pool(name="aggsb", bufs=2))
        tmpp = ctx.enter_context(tc.tile_pool(name="tmpp", bufs=2))
        rowp = ctx.enter_context(tc.tile_pool(name="rowp", bufs=2))

        # resident tensors
        h_sb = const.tile([D, NCP], f32)
        iota_sb = const.tile([128, 128], f32)
        ident_sb = const.tile([128, 128], f32)
        wie_sb = const.tile([128, L * 3 * 128], f32)
        whh_sb = const.tile([128, 3 * 128], f32)
        bias_sb = const.tile([128, 4], f32)
        rel_sb = const.tile([128, NCELLC * T], f32)
        ew_sb = const.tile([128, NCELLC * T], f32)

        nc.sync.dma_start(h_sb[:], x_ownT[:])
        nc.sync.dma_start(iota_sb[:], iota_dram[:])
        nc.sync.dma_start(ident_sb[:], ident_dram[:])
        nc.sync.dma_start(wie_sb[:], wie_dram[:])
        nc.sync.dma_start(whh_sb[:], whh_dram[:])
        nc.sync.dma_start(bias_sb[:], bias_dram[:])
        nc.sync.dma_start(rel_sb[:], rel_dram[:])
        nc.sync.dma_start(ew_sb[:], ewc_dram[:])

        # manually managed per-(sb, parity) message buffers
        msg_bufs = [[const.tile([128, spc_w], f32, name=f"msg{s}_{i}")
                     for i in range(2)] for s in range(SB)]

        h_bounce = [dram.tile([NCP, D], f32, name=f"h_bounce{l}") for l in range(2)]
        h_full = [dram.tile([TROWS, D], f32, name=f"h_full{l}", addr_space="Shared")
                  for l in range(2)]

        def wie(l, k):
            o = (l * 3 + k) * 128
            return wie_sb[:, o:o + 128]

        def whh(k):
            return whh_sb[:, k * 128:(k + 1) * 128]

        for l in range(L):
            table = x_tab if l == 0 else h_full[l - 1]
            for w in range(NW):
                # stream this window's idx block (4 units of spc_w slots)
                icols = SB * spc_w // 16
                idxt = idxp.tile([128, icols], i16, tag="idx")
                nc.sync.dma_start(idxt[:], idx_dram[:, w * icols:(w + 1) * icols])
                for sb in range(SB):
                    msg = msg_bufs[sb][w % 2]
                    gs = spc_w // GSPLIT
                    for gi in range(GSPLIT):
                        nc.gpsimd.dma_gather(
                            msg[:, gi * gs:(gi + 1) * gs].rearrange(
                                "p (t f) -> p t f", f=D),
                            table[sb * SBROWS:(sb + 1) * SBROWS, :],
                            idxt[:, (sb * spc_w + gi * gs) // 16:
                                 (sb * spc_w + (gi + 1) * gs) // 16],
                            gs, gs, D,
                            queue_num=sb % NQUEUES,
                        )
                pa = aggps.tile([128, WIN], f32, tag="agg")
                for g in range(G):
                    for sb in range(SB):
                        msg = msg_bufs[sb][w % 2]
                        for t in range(T):
                            col = ((w * G + g) * SB + sb) * T + t
                            S = sp.tile([128, 128], f32, tag="S")
                            nc.vector.tensor_scalar(
                                S[:], iota_sb[:],
                                rel_sb[:, col:col + 1], ew_sb[:, col:col + 1],
                                op0=eq, op1=mult)
                            moff = (g * T + t) * 128
                            nc.tensor.matmul(
                                pa[:, g * 128:(g + 1) * 128],
                                lhsT=msg[:, moff:moff + 128],
                                rhs=S[:],
                                start=(sb == 0 and t == 0),
                                stop=(sb == SB - 1 and t == T - 1),
                            )
                aggw = aggsb.tile([128, WIN], f32, tag="aggw")
                nc.scalar.activation(aggw[:], pa[:], AF.Copy)

                # ---- GRU chunk w ----
                cs = slice(w * WIN, (w + 1) * WIN)
                p_r = grups.tile([128, WIN], f32, tag="p_r")
                p_z = grups.tile([128, WIN], f32, tag="p_z")
                p_in = grups.tile([128, WIN], f32, tag="p_in")
                p_hn = grups.tile([128, WIN], f32, tag="p_hn")
                nc.tensor.matmul(p_r[:], lhsT=wie(l, 0), rhs=aggw[:], start=True, stop=False)
                nc.tensor.matmul(p_r[:], lhsT=whh(0), rhs=h_sb[:, cs], start=False, stop=True)
                nc.tensor.matmul(p_z[:], lhsT=wie(l, 1), rhs=aggw[:], start=True, stop=False)
                nc.tensor.matmul(p_z[:], lhsT=whh(1), rhs=h_sb[:, cs], start=False, stop=True)
                nc.tensor.matmul(p_in[:], lhsT=wie(l, 2), rhs=aggw[:], start=True, stop=True)
                nc.tensor.matmul(p_hn[:], lhsT=whh(2), rhs=h_sb[:, cs], start=True, stop=True)

                r_ = tmpp.tile([128, WIN], f32, tag="r")
                nc.scalar.activation(r_[:], p_r[:], AF.Sigmoid, bias=bias_sb[:, 0:1])
                z_ = tmpp.tile([128, WIN], f32, tag="z")
                nc.scalar.activation(z_[:], p_z[:], AF.Sigmoid, bias=bias_sb[:, 1:2])
                hnb = tmpp.tile([128, WIN], f32, tag="hnb")
                nc.scalar.activation(hnb[:], p_hn[:], AF.Identity, bias=bias_sb[:, 3:4])
                rt = tmpp.tile([128, WIN], f32, tag="rt")
                nc.vector.tensor_mul(rt[:], r_[:], hnb[:])
                s_ = tmpp.tile([128, WIN], f32, tag="s_")
                nc.vector.tensor_add(s_[:], p_in[:], rt[:])
                n_ = tmpp.tile([128, WIN], f32, tag="n_")
                nc.scalar.activation(n_[:], s_[:], AF.Tanh, bias=bias_sb[:, 2:3])
                d_ = tmpp.tile([128, WIN], f32, tag="d_")
                nc.vector.tensor_sub(d_[:], h_sb[:, cs], n_[:])
                zd = tmpp.tile([128, WIN], f32, tag="zd")
                nc.vector.tensor_mul(zd[:], z_[:], d_[:])
                nc.vector.tensor_add(h_sb[:, cs], n_[:], zd[:])

                # transpose h chunk to row-major and store
                p_t = grups.tile([128, WIN], f32, tag="p_t")
                for q in range(G):
                    nc.tensor.transpose(
                        p_t[:, q * 128:(q + 1) * 128],
                        h_sb[:, w * WIN + q * 128: w * WIN + (q + 1) * 128],
                        ident_sb[:])
                hr = rowp.tile([128, WIN], f32, tag="hr")
                nc.scalar.activation(hr[:], p_t[:], AF.Copy)
                dst = h_bounce[l] if l < 2 else out
                dview = dst[w * WIN:(w + 1) * WIN, :].rearrange(
                    "(q p) f -> p q f", p=128)
                nc.sync.dma_start(dview, hr.rearrange("p (q f) -> p q f", f=D))

            if l < 2:
                nc.gpsimd.collective_compute(
                    "AllGather",
                    mybir.AluOpType.bypass,
                    replica_groups=[list(range(C))],
                    ins=[h_bounce[l].opt()],
                    outs=[h_full[l].opt()],
                )

    nc.compile()
    return nc


# ---------------------------------------------------------------------------
# host wrappers
# ---------------------------------------------------------------------------
def _make_inputs(x, W, W_ih, W_hh, b_ih, b_hh, T, idx16, relcols, ewcols, pos):
    x = np.asarray(x, dtype=np.float32)
    W = np.asarray(W, dtype=np.float32)
    W_ih = np.asarray(W_ih, dtype=np.float32)
    W_hh = np.asarray(W_hh, dtype=np.float32)
    b_ih = np.asarray(b_ih, dtype=np.float32)
    b_hh = np.asarray(b_hh, dtype=np.float32)

    # permuted gather table for layer 0
    rows = (np.arange(N) // NC_) * NCP + pos
    x_tab = np.zeros((TROWS, D), dtype=np.float32)
    x_tab[rows] = x

    # Wi_eff_l = W_ih @ W_l.T ; lhsT chunk (l,k): Wi_eff_l[k*128:(k+1)*128, :].T
    wie = np.zeros((128, L * 3 * 128), dtype=np.float32)
    for l in range(L):
        wi = W_ih @ W[l].T  # [3D, D]
        for k in range(3):
            wie[:, (l * 3 + k) * 128:(l * 3 + k + 1) * 128] = wi[k * 128:(k + 1) * 128, :].T
    whh = np.zeros((128, 3 * 128), dtype=np.float32)
    for k in range(3):
        whh[:, k * 128:(k + 1) * 128] = W_hh[k * 128:(k + 1) * 128, :].T
    bias = np.zeros((128, 4), dtype=np.float32)
    bias[:, 0] = b_ih[0:128] + b_hh[0:128]
    bias[:, 1] = b_ih[128:256] + b_hh[128:256]
    bias[:, 2] = b_ih[256:384]
    bias[:, 3] = b_hh[256:384]

    iota = np.tile(np.arange(128, dtype=np.float32), (128, 1))
    ident = np.eye(128, dtype=np.float32)

    in_maps = []
    for c in range(C):
        pl = pos[c * NC_:(c + 1) * NC_]
        x_ownT = np.zeros((D, NCP), dtype=np.float32)
        x_ownT[:, pl] = x[c * NC_:(c + 1) * NC_].T
        in_maps.append({
            "x_tab": x_tab,
            "x_ownT": x_ownT,
            "idx_dram": np.ascontiguousarray(np.tile(idx16[c], (8, 1))),
            "rel_dram": np.ascontiguousarray(relcols[c]),
            "ewc_dram": np.ascontiguousarray(ewcols[c]),
            "wie_dram": wie,
            "whh_dram": whh,
            "bias_dram": bias,
            "iota_dram": np.ascontiguousarray(iota),
            "ident_dram": ident,
        })
    return in_maps


def _postprocess(res, pos):
    out = np.empty((N, D), dtype=np.float32)
    for c in range(C):
        out[c * NC_:(c + 1) * NC_] = res.results[c]["out"][pos[c * NC_:(c + 1) * NC_]]
    return out


_cache = {}


def kernel(x, edge_index, edge_attr, W, W_ih, W_hh, b_ih, b_hh):
    from concourse import bass_utils

    T, idx16, relcols, ewcols, pos = _plan(edge_index, edge_attr)
    if T not in _cache:
        _cache[T] = _build_program(T)
    nc = _cache[T]

    in_maps = _make_inputs(x, W, W_ih, W_hh, b_ih, b_hh,
                           T, idx16, relcols, ewcols, pos)
    res = bass_utils.run_bass_kernel_spmd(nc, in_maps, list(range(C)))
    return _postprocess(res, pos)


# revision 14
# speedup vs baseline: 1.0452x; 1.0452x over previous
"""GatedGraphConv (3-layer, GRU) Bass kernel for 8 Trainium2 NeuronCores. v3.

Strategy:
  - Shard nodes (dst segments) across 8 cores; each core owns 12500 dst nodes
    placed into 12800 padded positions via a host-chosen PERMUTATION that
    load-balances edges across (window, group, superblock) cells so every
    cell holds <= T*128 edges (T=4) with minimal padding.
  - fp32 data path end-to-end (the grading metric's +1e-3 denominator floor
    demands ~1e-5 absolute accuracy; 16-bit anywhere fails).
  - Messages gathered with ONE dma_gather per (window, superblock) =
    100 calls/layer of 2048 rows each (129 descs/engine-ring, under the 256
    ring capacity), spread across all 4 SWDGE queues so descriptor
    generation runs on all 8 GpSimd Q7 cores concurrently.
  - All padded slots gather row 0 with edge-weight 0 (no cnt registers).
  - Segment sums via TensorE matmuls with on-the-fly one-hot S matrices:
    S[slot, j] = ew[slot] * (dst_rel[slot] == j), accumulated in fp32 PSUM.
  - GRU per 512-node chunk interleaved with aggregation windows; elementwise
    split across ACT (sigmoid/tanh/copies/bias-adds) and DVE.
  - W_l folded into W_ih on the host (Wi_eff = W_ih @ W_l.T).
  - Updated h transposed back to row-major (PE), written to a bounce buffer,
    AllGathered so every core holds the full h table for the next layer.
  - Final layer writes fp32 permuted rows; host un-permutes.
"""

import sys
import numpy as np

for _p in ("/opt/trn_rl_repo",):
    if _p not in sys.path:
        sys.path.append(_p)

# ---------------------------------------------------------------------------
# constants (hardcoded problem shape)
# ---------------------------------------------------------------------------
N = 100000          # nodes
D = 128             # feature dim
L = 3               # layers
C = 8               # cores
NC_ = N // C        # real nodes per core (12500)
NCP = 12800         # padded positions per core
WIN = 512           # dst nodes per PSUM window
NW = NCP // WIN     # windows per core (25)
G = WIN // 128      # 128-wide subgroups per window (4)
SB = 4              # superblocks of gather table (int16 index limit)
TROWS = C * NCP     # padded gather-table rows (102400)
SBROWS = TROWS // SB  # rows per superblock (25600)
NQUEUES = 4         # SWDGE queues for parallel gather desc-gen
GSPLIT = 2          # gathers per (window, superblock) unit
NCELLC = NW * G * SB  # cells per core (400)
NBIN = NW * G       # dst bins per core (100)


def _ceil_div(a, b):
    return -(-a // b)


# ---------------------------------------------------------------------------
# host-side planning
# ---------------------------------------------------------------------------
def _plan(edge_index, edge_attr):
    """Balance dst nodes into bins, build gather indices + S-build scalars."""
    src = np.asarray(edge_index[0], dtype=np.int64)
    dst = np.asarray(edge_index[1], dtype=np.int64)
    ew = np.asarray(edge_attr, dtype=np.float32)
    E = src.shape[0]

    sb_e = src // (2 * NC_)  # source superblock, permutation-independent

    # per-(dst node, sb) edge counts
    cnt_ns = np.bincount(dst * SB + sb_e, minlength=N * SB).reshape(N, SB)

    # --- greedy multi-dim balance of dst nodes into NBIN bins per core ---
    pos = np.empty(N, dtype=np.int64)  # position of each node within its core
    maxcell = 0
    for c in range(C):
        v = cnt_ns[c * NC_:(c + 1) * NC_]  # [12500, 4]
        order = np.argsort(-v.sum(1), kind="stable")
        load = np.zeros((NBIN, SB), dtype=np.int64)
        nn = np.zeros(NBIN, dtype=np.int64)
        binof = np.empty(NC_, dtype=np.int64)
        for j in order:
            score = (load + v[j]).max(1).astype(np.float64) + nn * 1e-3
            score[nn >= 128] = np.inf
            b = int(np.argmin(score))
            binof[j] = b
            load[b] += v[j]
            nn[b] += 1
        o2 = np.argsort(binof, kind="stable")
        bsort = binof[o2]
        cnts = np.bincount(bsort, minlength=NBIN)
        starts = np.concatenate(([0], np.cumsum(cnts)[:-1]))
        ranks = np.arange(NC_) - starts[bsort]
        pl = np.empty(NC_, dtype=np.int64)
        pl[o2] = bsort * 128 + ranks
        pos[c * NC_:(c + 1) * NC_] = pl
        maxcell = max(maxcell, int(load.max()))

    T = _ceil_div(max(1, maxcell), 128)
    spc_w = G * T * 128  # slots per (window, sb) gather unit

    # --- per-edge slot assignment ---
    dc = dst // NC_
    posd = pos[dst]
    w = posd // WIN
    g = (posd % WIN) // 128
    rel = (posd % 128).astype(np.float32)
    srow_local = (src // NC_) * NCP + pos[src] - sb_e * SBROWS  # [0, 25600)

    cell = ((dc * NW + w) * G + g) * SB + sb_e
    order_e = np.argsort(cell, kind="stable")
    cell_s = cell[order_e]
    ccnt = np.bincount(cell_s, minlength=C * NCELLC)
    starts = np.concatenate(([0], np.cumsum(ccnt)[:-1]))
    rank = np.arange(E) - starts[cell_s]
    rank_e = np.empty(E, dtype=np.int64)
    rank_e[order_e] = rank
    assert int(ccnt.max()) <= T * 128

    # unit (w, sb) at slot base (w*SB + sb) * spc_w; in-unit slot = g*T*128+rank
    slot = (w * SB + sb_e) * spc_w + g * T * 128 + rank_e
    IDXCOLS = NW * SB * spc_w // 16

    idx16 = np.zeros((C, 16, IDXCOLS), dtype=np.int16)
    idx16[dc, slot % 16, slot // 16] = srow_local.astype(np.int16)

    t_e = rank_e // 128
    p_e = rank_e % 128
    colc = ((w * G + g) * SB + sb_e) * T + t_e
    relcols = np.zeros((C, 128, NCELLC * T), dtype=np.float32)
    ewcols = np.zeros((C, 128, NCELLC * T), dtype=np.float32)
    relcols[dc, p_e, colc] = rel
    ewcols[dc, p_e, colc] = ew

    return T, idx16, relcols, ewcols, pos


# ---------------------------------------------------------------------------
# device program
# ---------------------------------------------------------------------------
def _build_program(T):
    from contextlib import ExitStack
    import concourse.bass as bass
    import concourse.tile as tile
    from concourse import bacc, mybir

    f32 = mybir.dt.float32
    i16 = mybir.dt.int16
    eq = mybir.AluOpType.is_equal
    mult = mybir.AluOpType.mult
    AF = mybir.ActivationFunctionType

    spc_w = G * T * 128
    IDXCOLS = NW * SB * spc_w // 16

    nc = bacc.Bacc("TRN2", target_bir_lowering=False, debug=False,
                   num_devices=C, num_swdge_queues=NQUEUES)

    x_tab = nc.dram_tensor("x_tab", [TROWS, D], f32, kind="ExternalInput").ap()
    x_ownT = nc.dram_tensor("x_ownT", [D, NCP], f32, kind="ExternalInput").ap()
    idx_dram = nc.dram_tensor("idx_dram", [128, IDXCOLS], i16,
                              kind="ExternalInput").ap()
    rel_dram = nc.dram_tensor("rel_dram", [128, NCELLC * T], f32,
                              kind="ExternalInput").ap()
    ewc_dram = nc.dram_tensor("ewc_dram", [128, NCELLC * T], f32,
                              kind="ExternalInput").ap()
    wie_dram = nc.dram_tensor("wie_dram", [128, L * 3 * 128], f32, kind="ExternalInput").ap()
    whh_dram = nc.dram_tensor("whh_dram", [128, 3 * 128], f32, kind="ExternalInput").ap()
    bias_dram = nc.dram_tensor("bias_dram", [128, 4], f32, kind="ExternalInput").ap()
    iota_dram = nc.dram_tensor("iota_dram", [128, 128], f32, kind="ExternalInput").ap()
    ident_dram = nc.dram_tensor("ident_dram", [128, 128], f32, kind="ExternalInput").ap()

    out = nc.dram_tensor("out", [NCP, D], f32, kind="ExternalOutput").ap()

    with tile.TileContext(nc) as tc, ExitStack() as ctx:
        const = ctx.enter_context(tc.tile_pool(name="const", bufs=1))
        dram = ctx.enter_context(tc.tile_pool(name="dram", bufs=1, space="DRAM"))
        idxp = ctx.enter_context(tc.tile_pool(name="idxp", bufs=2))
        sp = ctx.enter_context(tc.tile_pool(name="sp", bufs=8))
        aggps = ctx.enter_context(tc.tile_pool(name="aggps", bufs=2, space="PSUM"))
        grups = ctx.enter_context(tc.tile_pool(name="grups", bufs=1, space="PSUM"))
        aggsb = ctx.enter_context(tc.tile_pool(name="aggsb", bufs=2))
        tmpp = ctx.enter_context(tc.tile_pool(name="tmpp", bufs=2))
        rowp = ctx.enter_context(tc.tile_pool(name="rowp", bufs=2))

        # resident tensors
        h_sb = const.tile([D, NCP], f32)
        iota_sb = const.tile([128, 128], f32)
        ident_sb = const.tile([128, 128], f32)
        wie_sb = const.tile([128, L * 3 * 128], f32)
        whh_sb = const.tile([128, 3 * 128], f32)
        bias_sb = const.tile([128, 4], f32)
        rel_sb = const.tile([128, NCELLC * T], f32)
        ew_sb = const.tile([128, NCELLC * T], f32)

        nc.sync.dma_start(h_sb[:], x_ownT[:])
        nc.sync.dma_start(iota_sb[:], iota_dram[:])
        nc.sync.dma_start(ident_sb[:], ident_dram[:])
        nc.sync.dma_start(wie_sb[:], wie_dram[:])
        nc.sync.dma_start(whh_sb[:], whh_dram[:])
        nc.sync.dma_start(bias_sb[:], bias_dram[:])
        nc.sync.dma_start(rel_sb[:], rel_dram[:])
        nc.sync.dma_start(ew_sb[:], ewc_dram[:])

        # manually managed per-(sb, parity) message buffers
        msg_bufs = [[const.tile([128, spc_w], f32, name=f"msg{s}_{i}")
                     for i in range(2)] for s in range(SB)]

        h_bounce = [dram.tile([NCP, D], f32, name=f"h_bounce{l}") for l in range(2)]
        h_full = [dram.tile([TROWS, D], f32, name=f"h_full{l}", addr_space="Shared")
                  for l in range(2)]

        def wie(l, k):
            o = (l * 3 + k) * 128
            return wie_sb[:, o:o + 128]

        def whh(k):
            return whh_sb[:, k * 128:(k + 1) * 128]

        for l in range(L):
            table = x_tab if l == 0 else h_full[l - 1]
            for w in range(NW):
                # stream this window's idx block (4 units of spc_w slots)
                icols = SB * spc_w // 16
                idxt = idxp.tile([128, icols], i16, tag="idx")
                nc.sync.dma_start(idxt[:], idx_dram[:, w * icols:(w + 1) * icols])
                for sb in range(SB):
                    msg = msg_bufs[sb][w % 2]
                    gs = spc_w // GSPLIT
                    for gi in range(GSPLIT):
                        nc.gpsimd.dma_gather(
                            msg[:, gi * gs:(gi + 1) * gs].rearrange(
                                "p (t f) -> p t f", f=D),
                            table[sb * SBROWS:(sb + 1) * SBROWS, :],
                            idxt[:, (sb * spc_w + gi * gs) // 16:
                                 (sb * spc_w + (gi + 1) * gs) // 16],
                            gs, gs, D,
                            queue_num=(sb * GSPLIT + gi) % NQUEUES,
                        )
                pa = aggps.tile([128, WIN], f32, tag="agg")
                for g in range(G):
                    for sb in range(SB):
                        msg = msg_bufs[sb][w % 2]
                        for t in range(T):
                            col = ((w * G + g) * SB + sb) * T + t
                            S = sp.tile([128, 128], f32, tag="S")
                            nc.vector.tensor_scalar(
                                S[:], iota_sb[:],
                                rel_sb[:, col:col + 1], ew_sb[:, col:col + 1],
                                op0=eq, op1=mult)
                            moff = (g * T + t) * 128
                            nc.tensor.matmul(
                                pa[:, g * 128:(g + 1) * 128],
                                lhsT=msg[:, moff:moff + 128],
                                rhs=S[:],
                                start=(sb == 0 and t == 0),
                                stop=(sb == SB - 1 and t == T - 1),
                            )
                aggw = aggsb.tile([128, WIN], f32, tag="aggw")
                nc.scalar.activation(aggw[:], pa[:], AF.Copy)

                # ---- GRU chunk w ----
                cs = slice(w * WIN, (w + 1) * WIN)
                p_r = grups.tile([128, WIN], f32, tag="p_r")
                p_z = grups.tile([128, WIN], f32, tag="p_z")
                p_in = grups.tile([128, WIN], f32, tag="p_in")
                p_hn = grups.tile([128, WIN], f32, tag="p_hn")
                nc.tensor.matmul(p_r[:], lhsT=wie(l, 0), rhs=aggw[:], start=True, stop=False)
                nc.tensor.matmul(p_r[:], lhsT=whh(0), rhs=h_sb[:, cs], start=False, stop=True)
                nc.tensor.matmul(p_z[:], lhsT=wie(l, 1), rhs=aggw[:], start=True, stop=False)
                nc.tensor.matmul(p_z[:], lhsT=whh(1), rhs=h_sb[:, cs], start=False, stop=True)
                nc.tensor.matmul(p_in[:], lhsT=wie(l, 2), rhs=aggw[:], start=True, stop=True)
                nc.tensor.matmul(p_hn[:], lhsT=whh(2), rhs=h_sb[:, cs], start=True, stop=True)

                r_ = tmpp.tile([128, WIN], f32, tag="r")
                nc.scalar.activation(r_[:], p_r[:], AF.Sigmoid, bias=bias_sb[:, 0:1])
                z_ = tmpp.tile([128, WIN], f32, tag="z")
                nc.scalar.activation(z_[:], p_z[:], AF.Sigmoid, bias=bias_sb[:, 1:2])
                hnb = tmpp.tile([128, WIN], f32, tag="hnb")
                nc.scalar.activation(hnb[:], p_hn[:], AF.Identity, bias=bias_sb[:, 3:4])
                rt = tmpp.tile([128, WIN], f32, tag="rt")
                nc.vector.tensor_mul(rt[:], r_[:], hnb[:])
                s_ = tmpp.tile([128, WIN], f32, tag="s_")
                nc.vector.tensor_add(s_[:], p_in[:], rt[:])
                n_ = tmpp.tile([128, WIN], f32, tag="n_")
                nc.scalar.activation(n_[:], s_[:], AF.Tanh, bias=bias_sb[:, 2:3])
                d_ = tmpp.tile([128, WIN], f32, tag="d_")
                nc.vector.tensor_sub(d_[:], h_sb[:, cs], n_[:])
                zd = tmpp.tile([128, WIN], f32, tag="zd")
                nc.vector.tensor_mul(zd[:], z_[:], d_[:])
                nc.vector.tensor_add(h_sb[:, cs], n_[:], zd[:])

                # transpose h chunk to row-major and store
                p_t = grups.tile([128, WIN], f32, tag="p_t")
                for q in range(G):
                    nc.tensor.transpose(
                        p_t[:, q * 128:(q + 1) * 128],
                        h_sb[:, w * WIN + q * 128: w * WIN + (q + 1) * 128],
                        ident_sb[:])
                hr = rowp.tile([128, WIN], f32, tag="hr")
                nc.scalar.activation(hr[:], p_t[:], AF.Copy)
                dst = h_bounce[l] if l < 2 else out
                dview = dst[w * WIN:(w + 1) * WIN, :].rearrange(
                    "(q p) f -> p q f", p=128)
                nc.sync.dma_start(dview, hr.rearrange("p (q f) -> p q f", f=D))

            if l < 2:
                nc.gpsimd.collective_compute(
                    "AllGather",
                    mybir.AluOpType.bypass,
                    replica_groups=[list(range(C))],
                    ins=[h_bounce[l].opt()],
                    outs=[h_full[l].opt()],
                )

    nc.compile()
    return nc


# ---------------------------------------------------------------------------
# host wrappers
# ---------------------------------------------------------------------------
def _make_inputs(x, W, W_ih, W_hh, b_ih, b_hh, T, idx16, relcols, ewcols, pos):
    x = np.asarray(x, dtype=np.float32)
    W = np.asarray(W, dtype=np.float32)
    W_ih = np.asarray(W_ih, dtype=np.float32)
    W_hh = np.asarray(W_hh, dtype=np.float32)
    b_ih = np.asarray(b_ih, dtype=np.float32)
    b_hh = np.asarray(b_hh, dtype=np.float32)

    # permuted gather table for layer 0
    rows = (np.arange(N) // NC_) * NCP + pos
    x_tab = np.zeros((TROWS, D), dtype=np.float32)
    x_tab[rows] = x

    # Wi_eff_l = W_ih @ W_l.T ; lhsT chunk (l,k): Wi_eff_l[k*128:(k+1)*128, :].T
    wie = np.zeros((128, L * 3 * 128), dtype=np.float32)
    for l in range(L):
        wi = W_ih @ W[l].T  # [3D, D]
        for k in range(3):
            wie[:, (l * 3 + k) * 128:(l * 3 + k + 1) * 128] = wi[k * 128:(k + 1) * 128, :].T
    whh = np.zeros((128, 3 * 128), dtype=np.float32)
    for k in range(3):
        whh[:, k * 128:(k + 1) * 128] = W_hh[k * 128:(k + 1) * 128, :].T
    bias = np.zeros((128, 4), dtype=np.float32)
    bias[:, 0] = b_ih[0:128] + b_hh[0:128]
    bias[:, 1] = b_ih[128:256] + b_hh[128:256]
    bias[:, 2] = b_ih[256:384]
    bias[:, 3] = b_hh[256:384]

    iota = np.tile(np.arange(128, dtype=np.float32), (128, 1))
    ident = np.eye(128, dtype=np.float32)

    in_maps = []
    for c in range(C):
        pl = pos[c * NC_:(c + 1) * NC_]
        x_ownT = np.zeros((D, NCP), dtype=np.float32)
        x_ownT[:, pl] = x[c * NC_:(c + 1) * NC_].T
        in_maps.append({
            "x_tab": x_tab,
            "x_ownT": x_ownT,
            "idx_dram": np.ascontiguousarray(np.tile(idx16[c], (8, 1))),
            "rel_dram": np.ascontiguousarray(relcols[c]),
            "ewc_dram": np.ascontiguousarray(ewcols[c]),
            "wie_dram": wie,
            "whh_dram": whh,
            "bias_dram": bias,
            "iota_dram": np.ascontiguousarray(iota),
            "ident_dram": ident,
        })
    return in_maps


def _postprocess(res, pos):
    out = np.empty((N, D), dtype=np.float32)
    for c in range(C):
        out[c * NC_:(c + 1) * NC_] = res.results[c]["out"][pos[c * NC_:(c + 1) * NC_]]
    return out


_cache = {}


def kernel(x, edge_index, edge_attr, W, W_ih, W_hh, b_ih, b_hh):
    from concourse import bass_utils

    T, idx16, relcols, ewcols, pos = _plan(edge_index, edge_attr)
    if T not in _cache:
        _cache[T] = _build_program(T)
    nc = _cache[T]

    in_maps = _make_inputs(x, W, W_ih, W_hh, b_ih, b_hh,
                           T, idx16, relcols, ewcols, pos)
    res = bass_utils.run_bass_kernel_spmd(nc, in_maps, list(range(C)))
    return _postprocess(res, pos)
